# revision 39
# baseline (speedup 1.0000x reference)
"""Trainium2 Bass kernel for the note/wiki 3-way contraction + gate MLP.

Math (per note n):
    e[n]    = (wikivec * notevec[n]) @ W_emb.T + b_emb          # (C, K)
    attn[n] = sigmoid(e[n] @ W_att.T + b_att)                   # (C, K)
    s[n]    = sum_k attn[n]*e[n]*W_out[0,k] + b_out             # (C,)

Sharding: data-parallel over the 16 notes -> 2 notes per core on 8 cores.

Phase 1 runs in fp8 e4m3 with DoubleRow perf mode (256-deep contraction per
pass, 0.5 PE cycles/out-col = 4x bf16 MACs/cycle). To make e4m3's ~4% relative
error survivable, the moving operand is the bilinear-centered product
ab = (notevec-1/2)*(wikivec-1/2) (3.5x smaller RMS than notevec*wikivec), and
the exact bilinear correction
    e = ab @ C^T + 0.25*sum_v C + 0.5*a@C^T + 0.5*b@C^T + b_emb
is computed on the host in fp32 and injected into the same PSUM accumulation
via 6 tiny bf16 matmuls with one-hot moving operands (per-c rows carry
0.5*b@C^T + 0.25*S0 + b_emb; per-note rows carry 0.5*a@C^T). Both fp8
operands are pre-scaled (ab by 64, W_emb by 16) so values stay in e4m3's
normal range; the 1024x product scale is divided out for free in the sigmoid's
scale argument and in a host-prescaled W_out.

Everything (ab8 5.2MB, C8 2.6MB per core) streams from HBM in graduated
blocks so the PE starts early and stays fed.
"""

import sys

if "/opt/trn_rl_repo" not in sys.path:
    sys.path.insert(0, "/opt/trn_rl_repo")

import numpy as np
import ml_dtypes

import concourse.bass as bass
import concourse.mybir as mybir
import concourse.tile as tile
from concourse import bacc
from concourse.bass_utils import run_bass_kernel_spmd

N_CORES = 8
N, C, V, K = 16, 256, 10000, 256
NLOC = N // N_CORES  # notes per core
NC2 = NLOC * C  # 512 (note, c) columns
DT = 40  # contraction dtiles of 256 v (V padded to 10240)
SUB = 2 * DT  # 80 sub-rows of 128 v each (s = 2*d + i)
VP = 128 * SUB  # 10240
SA = 64.0  # fp8 scale on the moving ab product
SC = 16.0  # fp8 scale on W_emb
S = SA * SC  # net scale on e held through phase 2

# uniform small data blocks, alternated across the two HWDGE queues
# (SP/ACT): many small transfers complete in consumption order, keeping
# PE stalls short — empirically better than big blocks despite per-block
# overhead. (start_dtile, len) per block.
AB_BLOCKS = [(0, 2), (2, 2)] + [(4 * i, 4) for i in range(1, 10)]
C8_BLOCKS = [(4 * i, 4) for i in range(10)]

F32 = mybir.dt.float32
BF16 = mybir.dt.bfloat16
F8 = mybir.dt.float8e4
BF16_NP = ml_dtypes.bfloat16
F8_NP = ml_dtypes.float8_e4m3

_NC_CACHE = {}


def _build_nc():
    nc = bacc.Bacc(None, target_bir_lowering=False)

    ab8 = nc.declare_dram_parameter("ab8", [128, SUB, NC2], F8, isOutput=False)
    c8 = nc.declare_dram_parameter("c8", [128, SUB, K], F8, isOutput=False)
    sbS = nc.declare_dram_parameter("sbS", [128, 2, K], BF16, isOutput=False)
    ohc = nc.declare_dram_parameter("ohc", [128, 2, NC2], BF16, isOutput=False)
    saT = nc.declare_dram_parameter("saT", [NLOC, 2, 128], BF16, isOutput=False)
    noh = nc.declare_dram_parameter("noh", [NLOC, NC2], BF16, isOutput=False)
    watT = nc.declare_dram_parameter("watT", [2, 128, K], BF16, isOutput=False)
    batt = nc.declare_dram_parameter("batt", [128, 2], F32, isOutput=False)
    woutT = nc.declare_dram_parameter("woutT", [128, NLOC], BF16, isOutput=False)
    bout2 = nc.declare_dram_parameter("bout2", [NLOC, 1], BF16, isOutput=False)
    s_out = nc.declare_dram_parameter("s_out", [1, NC2], F32, isOutput=True)

    with tile.TileContext(nc) as tc:
        with (
            tc.tile_pool(name="const", bufs=1) as constp,
            tc.tile_pool(name="c8p", bufs=1) as c8p,
            tc.tile_pool(name="abp", bufs=1) as abp,
            tc.tile_pool(name="post", bufs=1) as postp,
            tc.tile_pool(name="psum", bufs=1, space="PSUM") as psp,
        ):
            # ---- phase-1 accumulators: e^T[k-half, (note,c)] * S ----
            e_ps = [
                psp.tile([128, NC2], F32, name=f"e_ps{m}", tag=f"e_ps{m}")
                for m in range(2)
            ]

            # ---- DMAs: only SP and ACT queues have fast HWDGE (~215GB/s
            # each, ~430GB/s aggregate); gpsimd is SWDGE (~26GB/s), tiny
            # consts only. Uniform small blocks, alternated across the two
            # fast queues in consumption order, keep first-block latency low
            # and the in-flight window shallow. ----
            # tiny consts ride the slow gpsimd queue
            sat = constp.tile([NLOC, 2, 128], BF16)
            nc.gpsimd.dma_start(sat[:], saT[:])
            noht = constp.tile([NLOC, NC2], BF16)
            nc.gpsimd.dma_start(noht[:], noh[:])
            bo2 = constp.tile([NLOC, 1], BF16)
            nc.gpsimd.dma_start(bo2[:], bout2[:])
            bat = constp.tile([128, 2], F32)
            nc.gpsimd.dma_start(bat[:], batt[:])
            wout = constp.tile([128, NLOC], BF16)
            nc.gpsimd.dma_start(wout[:], woutT[:])
            wat = constp.tile([128, 2 * K], BF16)
            nc.gpsimd.dma_start(wat[:, 0:K], watT[0])
            nc.gpsimd.dma_start(wat[:, K : 2 * K], watT[1])

            # data blocks in consumption order; each 4-dtile super-step puts
            # the c8 block on one fast queue and the two ab8 blocks split so
            # both queues carry ~0.52MB per step. The last two c8 blocks are
            # needed late, so they ride the slow gpsimd queue, freeing
            # ~0.5MB of fast-queue bandwidth.
            # emit all data DMAs in consumption order, alternating queues
            sbt = constp.tile([128, 2, K], BF16)
            nc.scalar.dma_start(sbt[:], sbS[:])
            oht = constp.tile([128, 2, NC2], BF16)
            nc.scalar.dma_start(oht[:], ohc[:])
            # per 4-dtile super-step: the c8 block on one fast queue, the two
            # ab8 blocks split so both queues carry ~0.52MB per step; the last
            # two (late-needed) c8 blocks ride the slow gpsimd queue
            c8_q = [
                (nc.scalar if i % 2 == 0 else nc.sync)
                if i < len(C8_BLOCKS) - 2
                else nc.gpsimd
                for i in range(len(C8_BLOCKS))
            ]
            ab_q = [
                nc.sync if i % 2 == 0 else nc.scalar
                for i in range(len(AB_BLOCKS))
            ]
            events = []  # (start_dtile, order, kind, idx)
            for i, (s, l) in enumerate(AB_BLOCKS):
                events.append((s, 1, "a", i))
            for i, (s, l) in enumerate(C8_BLOCKS):
                events.append((s, 0, "c", i))
            events.sort()
            cts = [None] * len(C8_BLOCKS)
            abts = [None] * len(AB_BLOCKS)
            for s, _, kind, i in events:
                if kind == "c":
                    st, ln = C8_BLOCKS[i]
                    ct = c8p.tile([128, 2 * ln, K], F8, name=f"c8t{i}")
                    c8_q[i].dma_start(ct[:], c8[:, 2 * st : 2 * (st + ln), :])
                    cts[i] = ct
                else:
                    st, ln = AB_BLOCKS[i]
                    at = abp.tile([128, 2 * ln, NC2], F8, name=f"abt{i}")
                    ab_q[i].dma_start(at[:], ab8[:, 2 * st : 2 * (st + ln), :])
                    abts[i] = at

            def _find(blocks, d):
                for i, (s, l) in enumerate(blocks):
                    if s <= d < s + l:
                        return i, d - s
                raise AssertionError(d)

            # warm the ACT sigmoid table + bat semaphore lane after all ACT
            # queue DMA issues: the 1.3us table load must not fire on the
            # phase-2 tail (the sigmoid set also contains Copy/Identity)
            warm0 = constp.tile([128, 1], F32)
            nc.scalar.activation(
                warm0[:],
                bat[:, 0:1],
                mybir.ActivationFunctionType.Sigmoid,
                bias=bat[:, 0:1],
                scale=1.0,
            )

            a_ps = [
                psp.tile([128, NC2], F32, name=f"a_ps{jm}", tag=f"a_ps{jm}")
                for jm in range(2)
            ]
            s_ps = psp.tile([1, NC2], F32, tag="s_ps")
            eb = [
                postp.tile([128, NC2], BF16, name="eb0", tag="eb0"),
                postp.tile([128, NC2], BF16, name="eb1", tag="eb1"),
            ]

            # ---- fp8 DoubleRow data matmuls, d-major (self-pacing with the
            # block DMAs); Sa + b_out corrections splice in mid-stream once
            # their consts have landed ----
            def dr_mm(d, m, stop=False):
                ci, co = _find(C8_BLOCKS, d)
                ai, ao = _find(AB_BLOCKS, d)
                ct = cts[ci]
                at = abts[ai]
                subc = 2 * co
                suba = 2 * ao
                nc.tensor.matmul(
                    e_ps[m][:],
                    ct[:, subc : subc + 2, m * 128 : (m + 1) * 128],
                    at[:, suba : suba + 2, :],
                    start=(d == 0),
                    stop=stop,
                    perf_mode=mybir.MatmulPerfMode.DoubleRow,
                )

            for d in range(DT - 4):
                for m in range(2):
                    dr_mm(d, m)
                if d == 1:
                    # Sb corrections fill the early DMA-pacing bubbles
                    for ch in range(2):
                        for m in range(2):
                            nc.tensor.matmul(
                                e_ps[m][:],
                                sbt[:, ch, m * 128 : (m + 1) * 128],
                                oht[:, ch, :],
                                start=False,
                                stop=False,
                            )
                if d == 15:
                    for m in range(2):
                        nc.tensor.matmul(
                            e_ps[m][:],
                            sat[:, m, :],
                            noht[:],
                            start=False,
                            stop=False,
                        )
                    nc.tensor.matmul(
                        s_ps[:], bo2[:], noht[:], start=True, stop=False
                    )

            # close bank 0 four dtiles early so eb0 + the kt0 logit matmuls
            # overlap the bank-1 tail
            for d in range(DT - 4, DT):
                dr_mm(d, 0, stop=(d == DT - 1))
            nc.vector.tensor_copy(eb[0][:], e_ps[0][:])
            for d in range(DT - 4, DT):
                dr_mm(d, 1, stop=(d == DT - 1))

            # ---- phase-2 tail ----
            nc.scalar.copy(eb[1][:], e_ps[1][:])
            for kt in range(2):
                for jm in range(2):
                    nc.tensor.matmul(
                        a_ps[jm][:],
                        wat[:, kt * K + jm * 128 : kt * K + (jm + 1) * 128],
                        eb[kt][:],
                        start=(kt == 0),
                        stop=(kt == 1),
                    )

            for jm in range(2):
                atn = postp.tile([128, NC2], F32, tag=f"atn{jm}")
                nc.scalar.activation(
                    atn[:],
                    a_ps[jm][:],
                    mybir.ActivationFunctionType.Sigmoid,
                    bias=bat[:, jm : jm + 1],
                    scale=1.0 / S,
                )
                v_jm = postp.tile([128, NC2], BF16, tag=f"v{jm}")
                nc.vector.tensor_mul(v_jm[:], atn[:], e_ps[jm][:])
                nc.tensor.matmul(
                    s_ps[:],
                    wout[:, jm : jm + 1],
                    v_jm[:],
                    start=False,
                    stop=(jm == 1),
                )
            s_sb = postp.tile([1, NC2], F32, tag="s_sb")
            nc.scalar.copy(s_sb[:], s_ps[:])
            nc.sync.dma_start(s_out[:], s_sb[:])

    nc.compile()
    return nc


def _get_nc():
    if "nc" not in _NC_CACHE:
        _NC_CACHE["nc"] = _build_nc()
    return _NC_CACHE["nc"]


def prep_inputs(notevec, wikivec, W_emb, b_emb, W_att, b_att, W_out, b_out):
    A = np.asarray(notevec, np.float32)
    B = np.asarray(wikivec, np.float32)
    Cw = np.asarray(W_emb, np.float32)
    b_emb = np.asarray(b_emb, np.float32)
    W_att = np.asarray(W_att, np.float32)
    b_att = np.asarray(b_att, np.float32)
    W_out = np.asarray(W_out, np.float32)
    b_out = np.asarray(b_out, np.float32)

    a = A - 0.5
    b = B - 0.5
    aP = np.zeros((N, VP), np.float32)
    aP[:, :V] = a
    bP = np.zeros((C, VP), np.float32)
    bP[:, :V] = b
    CP = np.zeros((K, VP), np.float32)
    CP[:, :V] = Cw

    # c8[p, s, k] = SC * C[k, 128*s + p]
    c8 = np.ascontiguousarray(
        (CP * SC).reshape(K, SUB, 128).transpose(2, 1, 0)
    ).astype(F8_NP)

    # bilinear correction pieces (exact fp32 on host)
    S0 = Cw.sum(axis=1)  # (K,)
    Sa_ = a @ Cw.T  # (N, K)
    Sb_ = b @ Cw.T  # (C, K)
    sb_full = S * (0.5 * Sb_ + 0.25 * S0[None, :] + b_emb[None, :])  # (C, K)
    # sbS[p, ch, k] = sb_full[128*ch + p, k]
    sbS = np.ascontiguousarray(sb_full.reshape(2, 128, K).transpose(1, 0, 2)).astype(
        BF16_NP
    )
    # ohc[p, ch, note*256 + c] = 1 if c == 128*ch + p
    cols_c = np.tile(np.arange(C), NLOC)  # c index per column
    ohc = np.zeros((128, 2, NC2), np.float32)
    for ch in range(2):
        ohc[:, ch, :] = (cols_c[None, :] == (128 * ch + np.arange(128)[:, None]))
    ohc = ohc.astype(BF16_NP)
    # noh[p, col] = 1 if note(col) == p
    cols_n = np.repeat(np.arange(NLOC), C)
    noh = (cols_n[None, :] == np.arange(NLOC)[:, None]).astype(BF16_NP)

    watT = np.ascontiguousarray(W_att.T.reshape(2, 128, K)).astype(BF16_NP)
    batT = np.ascontiguousarray(b_att.reshape(2, 128).T)
    woutT = np.ascontiguousarray(W_out[0].reshape(2, 128).T / S).astype(BF16_NP)
    # s_ps[col] += sum_p bout2[p]*noh[p,col] and noh is a note one-hot,
    # so each column picks up b_out exactly once
    bout2 = np.full((NLOC, 1), b_out[0], np.float32).astype(BF16_NP)

    in_maps = []
    for i in range(N_CORES):
        ab = aP[NLOC * i : NLOC * (i + 1), None, :] * bP[None, :, :]  # (2, C, VP)
        # ab8[p, s, note*256+c] = SA * ab[note, c, 128*s + p]
        ab8 = np.ascontiguousarray(
            (ab * SA).reshape(NLOC, C, SUB, 128).transpose(3, 2, 0, 1)
        ).reshape(128, SUB, NC2).astype(F8_NP)
        # saT[note, m, j] = S * 0.5 * Sa[2i+note, 128m + j]
        sa_core = np.ascontiguousarray(
            (S * 0.5 * Sa_[NLOC * i : NLOC * (i + 1)]).reshape(NLOC, 2, 128)
        ).astype(BF16_NP)
        in_maps.append(
            {
                "ab8": ab8,
                "c8": c8,
                "sbS": sbS,
                "ohc": ohc,
                "saT": sa_core,
                "noh": noh,
                "watT": watT,
                "batt": batT,
                "woutT": woutT,
                "bout2": bout2,
            }
        )
    return in_maps


def run(in_maps, **kw):
    nc = _get_nc()
    return run_bass_kernel_spmd(nc, in_maps, list(range(N_CORES)), **kw)


def kernel(notevec, wikivec, W_emb, b_emb, W_att, b_att, W_out, b_out):
    in_maps = prep_inputs(
        notevec, wikivec, W_emb, b_emb, W_att, b_att, W_out, b_out
    )
    res = run(in_maps)
    out = np.concatenate(
        [r["s_out"].reshape(NLOC, C) for r in res.results], axis=0
    )
    return out.astype(np.float32)
